# revision 7
# baseline (speedup 1.0000x reference)
"""Trainium2 Bass kernel for a dense multi-head self-attention block.

Computation (matches torch/diffusers Attention with upcast softmax):
    q/k/v = hs @ W.T + b ; per-head scaled QK^T ; softmax ; PV ; out proj.
Shapes: hs [2, 2048, 1024], 16 heads x 64 dim, fp32 in/out.

Sharding: batch*head parallel over 8 cores. Core c owns heads {2c, 2c+1}
(feature slice c*128:(c+1)*128 of E) for both batches. The host
pre-transposes hidden_states to [E, B*S] and pre-slices/transposes the
weights (fp16), so the device never transposes activations. Per core:
  - Q^T/K^T/V^T projections for its 128 features over all 4096 tokens
    (fp16 operands, fp32 PSUM accumulation),
  - V^T is re-tiled to [tokens, features] via PE transposes into per-head
    stationary tiles; an all-ones column rides along so the PV matmul also
    accumulates the softmax denominator,
  - attention in scores^T layout (K @ Q^T), one head at a time per
    (batch, 1024-wide q-block): QK fills a double-buffered [128,1024]
    PSUM score tile per 128-k-token step, exp runs on ScalarE straight
    out of PSUM with the 1/sqrt(d) scale folded in (no max-subtraction:
    scores are O(1) by construction), PV accumulates [*,512] PSUM tiles.
    Head 0's V tile carries values in columns 0:64 + ones in column 64
    (denominator lands on PSUM row 64); head 1's V tile carries ones in
    column 63 + values in columns 64:128 (denominator on row 63, values
    on rows 64:128) so the normalized output lands directly on SBUF
    partitions 64:128 with no partition-shift DMA.
  - softmax normalization: denominators are packed across 128 partitions
    via a DRAM bounce, reciprocal'd in one cheap DVE op, broadcast back
    with DMA broadcast-reads, then fused into the PSUM->SBUF multiply,
  - partial out-projection (contraction over this core's 128 features)
    written as fp16 [4096, 1024]; the host sums the 8 partials + o_b.

Pipelining: the whole kernel is emitted so the Tile dataflow scheduler
overlaps phases. Attention for batch 0 starts as soon as batch 0's QKV
is done; batch 1's QKV projections and all out-projections are emitted
AFTER the attention units they should hide under, so the (in-priority-
order) PE ready-heap treats them as filler for the ~200ns/step the PE
would otherwise idle while ScalarE (the phase drum, ~1.05us per 128k x
1024q exp) catches up. PSUM: score pool 4 banks + PV pool 2 banks +
filler pool (QKV/out-proj/transposes) 2 banks. ScalarE does exp ONLY;
every PSUM evacuation is on DVE; norm-bounce + output DMAs issue from
the otherwise-idle GpSimd queue.
"""

import numpy as np

import concourse.bass as bass
import concourse.mybir as mybir
import concourse.tile as tile
from concourse import bacc
from concourse.bass_utils import run_bass_kernel_spmd

B, S, E = 2, 2048, 1024
H, D = 16, 64
SCALE = D ** -0.5
NCORE = 8
T = B * S              # 4096 tokens
FPC = 128              # features per core (2 heads x 64)
HPC = 2                # heads per core

F32 = mybir.dt.float32
F16 = mybir.dt.float16
EXP = mybir.ActivationFunctionType.Exp

# set by test harness to profile; results stashed in LAST_RESULT
TRACE = False
DEBUG = False
LAST_RESULT = None
_CACHE = {}


def _build(ctx, tc, io):
    nc = tc.nc
    hs_t, wq_t, wk_t, wv_t, ow_t, out_p = (
        io["hs_t"], io["wq_t"], io["wk_t"], io["wv_t"], io["ow_t"], io["out_p"],
    )

    # ---------------- pools ----------------
    consts = ctx.enter_context(tc.tile_pool(name="consts", bufs=1))
    persist = ctx.enter_context(tc.tile_pool(name="persist", bufs=1))
    hst_pool = ctx.enter_context(tc.tile_pool(name="hst", bufs=4))
    vt_pool = ctx.enter_context(tc.tile_pool(name="vt", bufs=3))
    pt_pool = ctx.enter_context(tc.tile_pool(name="pt", bufs=6))
    bc_pool = ctx.enter_context(tc.tile_pool(name="bcs", bufs=2))
    rc_pool = ctx.enter_context(tc.tile_pool(name="rc", bufs=2))
    out_pool = ctx.enter_context(tc.tile_pool(name="outs", bufs=6))
    dr_pool = ctx.enter_context(tc.tile_pool(name="drb", bufs=4, space="DRAM"))
    # PSUM: 8 banks. p_sc = 2x[128,1024] (4 banks), p_pv = 2x[128,512]
    # (2 banks), p_fil = 2x[128,512] (2 banks, QKV/transpose/out-proj).
    p_sc = ctx.enter_context(tc.tile_pool(name="p_sc", bufs=2, space="PSUM"))
    p_pv = ctx.enter_context(tc.tile_pool(name="p_pv", bufs=2, space="PSUM"))
    p_fil = ctx.enter_context(tc.tile_pool(name="p_fil", bufs=2, space="PSUM"))

    # ---------------- constants / weights ----------------
    # DMA order matters for the pipeline head: bias+wq+first hs tile first
    # so the first projection matmuls can fire ASAP; ow (needed only ~100us
    # in, at the first out-projection) goes last.
    bias_sb = consts.tile([128, 3], F32, tag="bias")
    wq_sb = consts.tile([128, 8, 128], F16, tag="wq")
    wk_sb = consts.tile([128, 8, 128], F16, tag="wk")
    wv_sb = consts.tile([128, 8, 128], F16, tag="wv")
    cpack = consts.tile([128, 144], F16, tag="cpack")
    ow_sb = consts.tile([128, 1024], F16, tag="ow")
    ident = cpack[:, 0:128]
    qb_sb, kb_sb, vb_sb = bias_sb[:, 0:1], bias_sb[:, 1:2], bias_sb[:, 2:3]

    nc.sync.dma_start(bias_sb[:], io["bias3"][:])
    nc.sync.dma_start(wq_sb[:], wq_t.rearrange("(t p) m -> p t m", p=128))
    hst0 = hst_pool.tile([128, 8, 512], F16, tag="hst", name="hst0")
    nc.sync.dma_start(
        hst0[:, 0:4, :], hs_t[0:512, 0:512].rearrange("(t p) n -> p t n", p=128)
    )
    nc.sync.dma_start(
        hst0[:, 4:8, :], hs_t[512:1024, 0:512].rearrange("(t p) n -> p t n", p=128)
    )
    nc.sync.dma_start(wk_sb[:], wk_t.rearrange("(t p) m -> p t m", p=128))
    nc.sync.dma_start(wv_sb[:], wv_t.rearrange("(t p) m -> p t m", p=128))
    nc.sync.dma_start(cpack[:], io["cpack"][:])
    nc.sync.dma_start(ow_sb[:], ow_t[:])

    # persistent activations: feature dim (128 = 2 heads x 64) on partitions
    qt = [persist.tile([128, S], F16, tag=f"qt{b}", name=f"qt{b}") for b in range(B)]
    kt = [persist.tile([128, S], F16, tag=f"kt{b}", name=f"kt{b}") for b in range(B)]
    at = [
        [
            persist.tile([128, 1024], F16, tag=f"at{b}{qb}", name=f"at{b}{qb}")
            for qb in range(2)
        ]
        for b in range(B)
    ]
    # Per-head stationary V tiles; tile index = 128-k-token block.
    # v0[b][:, kt, 0:64] = head-0 values, [:, kt, 64] = ones (denom row 64).
    # v1[b][:, kt, 32] = ones (denom row 32 -- engine partition accesses
    # must be 32-aligned), [:, kt, 64:128] = head-1 values (output rows
    # 64:128), other columns zero.
    v0 = [
        persist.tile([128, 16, 65], F16, tag=f"v0{b}", name=f"v0{b}") for b in range(B)
    ]
    v1 = [
        persist.tile([128, 16, 128], F16, tag=f"v1{b}", name=f"v1{b}")
        for b in range(B)
    ]
    ones_col = cpack[:, 128:144].rearrange("p (a o) -> p a o", o=1)
    for b in range(B):
        nc.gpsimd.memset(v1[b][:, :, 0:64], 0.0)
        nc.gpsimd.tensor_copy(v0[b][:, :, 64:65], ones_col)
        nc.gpsimd.tensor_copy(v1[b][:, :, 32:33], ones_col)

    # ---------------- building blocks ----------------
    def qkv_block(tb, hst=None):
        """Q^T/K^T/V^T projections + V transposes for one 512-token block."""
        b, tl = tb // 4, tb % 4
        if hst is None:
            hst = hst_pool.tile([128, 8, 512], F16, tag="hst")
            nc.sync.dma_start(
                hst[:],
                hs_t[:, tb * 512:(tb + 1) * 512].rearrange("(t p) n -> p t n", p=128),
            )
        c0 = tl * 512
        for w_sb, b_ap, dest in ((wq_sb, qb_sb, qt[b]), (wk_sb, kb_sb, kt[b])):
            ps = p_fil.tile([128, 512], F32, tag="fil", name="ps")
            for et in range(8):
                nc.tensor.matmul(
                    ps[:], w_sb[:, et, :], hst[:, et, :],
                    start=(et == 0), stop=(et == 7),
                )
            nc.vector.tensor_scalar_add(dest[:, c0:c0 + 512], ps[:], b_ap)
        vps = p_fil.tile([128, 512], F32, tag="fil", name="vps")
        for et in range(8):
            nc.tensor.matmul(
                vps[:], wv_sb[:, et, :], hst[:, et, :],
                start=(et == 0), stop=(et == 7),
            )
        vtt = vt_pool.tile([128, 512], F16, tag="vtt")
        nc.vector.tensor_scalar_add(vtt[:], vps[:], vb_sb)
        for j in range(4):
            ktl = tl * 4 + j
            tps = p_fil.tile([128, 128], F16, tag="fil", name="tps")
            nc.tensor.transpose(tps[:], vtt[:, j * 128:(j + 1) * 128], ident[:])
            nc.vector.tensor_copy(v0[b][:, ktl, 0:64], tps[:, 0:64])
            nc.vector.tensor_copy(v1[b][:, ktl, 64:128], tps[:, 64:128])

    def attn_unit(b, qb, h):
        """Scores + exp + PV accumulation for one (batch, q-block, head).

        Returns the two PSUM PV accumulators (qs = 0, 1)."""
        p0 = h * 64
        qoff = qb * 1024
        v_sb = v0[b] if h == 0 else v1[b]
        mrows = 65 if h == 0 else 128

        def emit_qk(k2):
            sc = p_sc.tile([128, 1024], F32, tag="sc", name="sc")
            for qs in range(2):
                nc.tensor.matmul(
                    sc[:, qs * 512:(qs + 1) * 512],
                    kt[b][p0:p0 + 64, k2 * 128:(k2 + 1) * 128],
                    qt[b][p0:p0 + 64, qoff + qs * 512:qoff + (qs + 1) * 512],
                    start=True, stop=True,
                )
            return sc

        pvq = [
            p_pv.tile([mrows, 512], F32, tag="pv", name=f"pv{qs}") for qs in range(2)
        ]
        sc_next = emit_qk(0)
        for k2 in range(16):
            sc = sc_next
            pt = pt_pool.tile([128, 1024], F16, tag="pt")
            nc.scalar.activation(pt[:], sc[:], EXP, scale=SCALE)
            if k2 < 15:
                sc_next = emit_qk(k2 + 1)
            first, last = k2 == 0, k2 == 15
            for qs in range(2):
                nc.tensor.matmul(
                    pvq[qs][:], v_sb[:, k2, :],
                    pt[:, qs * 512:(qs + 1) * 512], start=first, stop=last,
                )
        return pvq

    def attn_norm(b, qb, pvh0, pvh1):
        """Evacuate both heads' PV accumulators, normalize into at[b][qb]."""
        # pvs layout: cols 0:1024 = head 0 (rows 0:64 values, row 64 denom),
        # cols 1024:2048 = head 1 (row 32 denom, rows 64:128 values).
        pvs = rc_pool.tile([128, 2048], F32, tag="pvs", name="pvs")
        for qs in range(2):
            nc.vector.tensor_copy(pvs[0:65, qs * 512:(qs + 1) * 512], pvh0[qs][:])
            nc.vector.tensor_copy(
                pvs[:, 1024 + qs * 512:1024 + (qs + 1) * 512], pvh1[qs][:]
            )
        # Reciprocal of the 2048 denominators (2 heads x 1024 q).
        # DVE reciprocal costs ~6.3 ns per free-dim element regardless of
        # partition count, so pack them across 128 partitions via a DRAM
        # bounce: 2x[1,1024] rows -> [128,16] -> recip -> rows -> broadcast.
        den_dr = dr_pool.tile([2, 1024], F32, tag="den_dr", name="den_dr")
        nc.gpsimd.dma_start(den_dr[0:1, :], pvs[64:65, 0:1024])
        nc.gpsimd.dma_start(den_dr[1:2, :], pvs[32:33, 1024:2048])
        dpack = rc_pool.tile([128, 16], F32, tag="dpack", name="dpack")
        nc.gpsimd.dma_start(
            dpack[:],
            den_dr.rearrange("a n -> (a n)").rearrange("(p i) -> p i", p=128),
        )
        rpack = rc_pool.tile([128, 16], F32, tag="rpack", name="rpack")
        with nc.allow_low_precision(reason="softmax denom reciprocal"):
            nc.vector.reciprocal(rpack[:], dpack[:])
        rcp_dr = dr_pool.tile([2, 1024], F32, tag="rcp_dr", name="rcp_dr")
        nc.gpsimd.dma_start(
            rcp_dr.rearrange("a n -> (a n)").rearrange("(p i) -> p i", p=128),
            rpack[:],
        )
        bc = bc_pool.tile([128, 1024], F32, tag="bcs", name="bc")
        nc.gpsimd.dma_start(bc[0:64, :], rcp_dr[0:1, :].broadcast_to([64, 1024]))
        nc.gpsimd.dma_start(bc[64:128, :], rcp_dr[1:2, :].broadcast_to([64, 1024]))
        nc.vector.tensor_mul(at[b][qb][0:64, :], pvs[0:64, 0:1024], bc[0:64, :])
        nc.vector.tensor_mul(
            at[b][qb][64:128, :], pvs[64:128, 1024:2048], bc[64:128, :]
        )

    def outproj(b, qb):
        """Partial out-projection for one q-block's 1024 tokens."""
        for tb in range(8):
            t0g = b * 2048 + qb * 1024 + tb * 128
            t0l = tb * 128
            ot = out_pool.tile([128, 1024], F16, tag="outs", name="ot")
            for eb in range(2):
                ops = p_fil.tile([128, 512], F32, tag="fil", name="ops")
                nc.tensor.matmul(
                    ops[:], at[b][qb][:, t0l:t0l + 128],
                    ow_sb[:, eb * 512:(eb + 1) * 512],
                    start=True, stop=True,
                )
                nc.vector.tensor_copy(ot[:, eb * 512:(eb + 1) * 512], ops[:])
            nc.gpsimd.dma_start(out_p[t0g:t0g + 128, :], ot[:])

    # ---------------- pipeline ----------------
    # Emission order = scheduler priority. Attention units lead; batch-1
    # QKV blocks and out-projections trail the units they hide under.
    qkv_block(0, hst0)
    qkv_block(1)
    qkv_block(2)
    qkv_block(3)

    pv_h = {}
    pv_h[0] = attn_unit(0, 0, 0)
    qkv_block(4)
    pv_h[1] = attn_unit(0, 0, 1)
    qkv_block(5)
    attn_norm(0, 0, pv_h[0], pv_h[1])
    pv_h[0] = attn_unit(0, 1, 0)
    qkv_block(6)
    pv_h[1] = attn_unit(0, 1, 1)
    qkv_block(7)
    attn_norm(0, 1, pv_h[0], pv_h[1])

    pv_h[0] = attn_unit(1, 0, 0)
    outproj(0, 0)
    pv_h[1] = attn_unit(1, 0, 1)
    outproj(0, 1)
    attn_norm(1, 0, pv_h[0], pv_h[1])
    pv_h[0] = attn_unit(1, 1, 0)
    outproj(1, 0)
    pv_h[1] = attn_unit(1, 1, 1)
    attn_norm(1, 1, pv_h[0], pv_h[1])
    outproj(1, 1)

    if DEBUG:
        for b in range(B):
            nc.sync.dma_start(io["dbg_qt"][:, b * S:(b + 1) * S], qt[b][:])
            nc.sync.dma_start(io["dbg_kt"][:, b * S:(b + 1) * S], kt[b][:])
            for qb in range(2):
                nc.sync.dma_start(
                    io["dbg_at"][:, b * S + qb * 1024:b * S + (qb + 1) * 1024],
                    at[b][qb][:],
                )
        nc.sync.dma_start(
            io["dbg_v00"][:], v0[0][:].rearrange("p a b -> p (a b)")
        )


def _get_program():
    if "nc" in _CACHE:
        return _CACHE["nc"]
    from contextlib import ExitStack

    nc = bacc.Bacc("TRN2", target_bir_lowering=False, debug=False,
                   num_devices=NCORE)
    io = {
        "hs_t": nc.dram_tensor("hs_t", [E, T], F16, kind="ExternalInput").ap(),
        "wq_t": nc.dram_tensor("wq_t", [E, FPC], F16, kind="ExternalInput").ap(),
        "wk_t": nc.dram_tensor("wk_t", [E, FPC], F16, kind="ExternalInput").ap(),
        "wv_t": nc.dram_tensor("wv_t", [E, FPC], F16, kind="ExternalInput").ap(),
        "ow_t": nc.dram_tensor("ow_t", [FPC, E], F16, kind="ExternalInput").ap(),
        "bias3": nc.dram_tensor("bias3", [FPC, 3], F32, kind="ExternalInput").ap(),
        "cpack": nc.dram_tensor("cpack", [128, 144], F16, kind="ExternalInput").ap(),
        "out_p": nc.dram_tensor("out_p", [T, E], F16, kind="ExternalOutput").ap(),
    }
    if DEBUG:
        io["dbg_qt"] = nc.dram_tensor("dbg_qt", [128, T], F16, kind="ExternalOutput").ap()
        io["dbg_kt"] = nc.dram_tensor("dbg_kt", [128, T], F16, kind="ExternalOutput").ap()
        io["dbg_at"] = nc.dram_tensor("dbg_at", [128, T], F16, kind="ExternalOutput").ap()
        io["dbg_v00"] = nc.dram_tensor("dbg_v00", [128, 16 * 65], F16, kind="ExternalOutput").ap()
    with tile.TileContext(nc) as tc:
        with ExitStack() as ctx:
            _build(ctx, tc, io)
    nc.compile()
    _CACHE["nc"] = nc
    return nc


def kernel(hidden_states, q_w, q_b, k_w, k_b, v_w, v_b, o_w, o_b):
    global LAST_RESULT
    nc = _get_program()

    f32c = lambda a: np.ascontiguousarray(a, dtype=np.float32)
    f16c = lambda a: np.ascontiguousarray(a, dtype=np.float16)
    hs_t = f16c(np.asarray(hidden_states, dtype=np.float32).reshape(T, E).T)
    in_maps = []
    for c in range(NCORE):
        sl = slice(c * FPC, (c + 1) * FPC)
        in_maps.append({
            "hs_t": hs_t,
            "wq_t": f16c(np.asarray(q_w)[sl, :].T),
            "wk_t": f16c(np.asarray(k_w)[sl, :].T),
            "wv_t": f16c(np.asarray(v_w)[sl, :].T),
            "ow_t": f16c(np.asarray(o_w)[:, sl].T),
            "bias3": f32c(np.stack([np.asarray(q_b)[sl], np.asarray(k_b)[sl],
                                     np.asarray(v_b)[sl]], axis=1)),
            "cpack": f16c(np.concatenate([np.eye(128, dtype=np.float16),
                                          np.ones((128, 16), np.float16)], axis=1)),
        })

    res = run_bass_kernel_spmd(nc, in_maps, list(range(NCORE)), trace=TRACE)
    LAST_RESULT = res
    out = res.results[0]["out_p"].astype(np.float64)
    for c in range(1, NCORE):
        out += res.results[c]["out_p"]
    out += np.asarray(o_b, dtype=np.float64)
    return out.reshape(B, S, E).astype(np.float32)


# revision 14
# speedup vs baseline: 1.0181x; 1.0181x over previous
"""Trainium2 Bass kernel for a dense multi-head self-attention block.

Computation (matches torch/diffusers Attention with upcast softmax):
    q/k/v = hs @ W.T + b ; per-head scaled QK^T ; softmax ; PV ; out proj.
Shapes: hs [2, 2048, 1024], 16 heads x 64 dim, fp32 in/out.

Sharding: batch*head parallel over 8 cores. Core c owns heads {2c, 2c+1}
(feature slice c*128:(c+1)*128 of E) for both batches. The host
pre-transposes hidden_states to [E, B*S] and pre-slices/transposes the
weights (fp16), so the device never transposes activations. Per core:
  - Q^T/K^T/V^T projections for its 128 features over all 4096 tokens
    (fp16 operands, fp32 PSUM accumulation),
  - V^T is re-tiled to [tokens, features] via PE transposes into per-head
    stationary tiles; an all-ones column rides along so the PV matmul also
    accumulates the softmax denominator,
  - attention in scores^T layout (K @ Q^T), one head at a time per
    (batch, 1024-wide q-block): QK fills a double-buffered [128,1024]
    PSUM score tile per 128-k-token step, exp runs on ScalarE straight
    out of PSUM with the 1/sqrt(d) scale folded in (no max-subtraction:
    scores are O(1) by construction), PV accumulates [*,512] PSUM tiles.
    Head 0's V tile carries values in columns 0:64 + ones in column 64
    (denominator lands on PSUM row 64); head 1's V tile carries ones in
    column 63 + values in columns 64:128 (denominator on row 63, values
    on rows 64:128) so the normalized output lands directly on SBUF
    partitions 64:128 with no partition-shift DMA.
  - softmax normalization: denominators are packed across 128 partitions
    via a DRAM bounce, reciprocal'd in one cheap DVE op, broadcast back
    with DMA broadcast-reads, then fused into the PSUM->SBUF multiply,
  - partial out-projection (contraction over this core's 128 features)
    written as fp16 [4096, 1024]; the host sums the 8 partials + o_b.

Pipelining: the whole kernel is emitted so the Tile dataflow scheduler
overlaps phases. Attention for batch 0 starts as soon as batch 0's QKV
is done; batch 1's QKV projections and all out-projections are emitted
AFTER the attention units they should hide under, so the (in-priority-
order) PE ready-heap treats them as filler for the ~200ns/step the PE
would otherwise idle while ScalarE (the phase drum, ~1.05us per 128k x
1024q exp) catches up. PSUM: score pool 4 banks + PV pool 2 banks +
filler pool (QKV/out-proj/transposes) 2 banks. ScalarE does exp ONLY;
every PSUM evacuation is on DVE; norm-bounce + output DMAs issue from
the otherwise-idle GpSimd queue.
"""

import numpy as np

import concourse.bass as bass
import concourse.mybir as mybir
import concourse.tile as tile
from concourse import bacc
from concourse.bass_utils import run_bass_kernel_spmd

B, S, E = 2, 2048, 1024
H, D = 16, 64
SCALE = D ** -0.5
NCORE = 8
T = B * S              # 4096 tokens
FPC = 128              # features per core (2 heads x 64)
HPC = 2                # heads per core

F32 = mybir.dt.float32
F16 = mybir.dt.float16
EXP = mybir.ActivationFunctionType.Exp

# set by test harness to profile; results stashed in LAST_RESULT
TRACE = False
DEBUG = False
LAST_RESULT = None
_CACHE = {}


def _build(ctx, tc, io):
    nc = tc.nc
    hs_t, wq_t, wk_t, wv_t, ow_t, out_p = (
        io["hs_t"], io["wq_t"], io["wk_t"], io["wv_t"], io["ow_t"], io["out_p"],
    )

    # ---------------- pools ----------------
    consts = ctx.enter_context(tc.tile_pool(name="consts", bufs=1))
    persist = ctx.enter_context(tc.tile_pool(name="persist", bufs=1))
    hst_pool = ctx.enter_context(tc.tile_pool(name="hst", bufs=4))
    vt_pool = ctx.enter_context(tc.tile_pool(name="vt", bufs=3))
    pt_pool = ctx.enter_context(tc.tile_pool(name="pt", bufs=8))
    rc_pool = ctx.enter_context(tc.tile_pool(name="rc", bufs=2))
    out_pool = ctx.enter_context(tc.tile_pool(name="outs", bufs=6))
    # PSUM: 8 banks. p_sc = 2x[128,1024] (4 banks), p_pv = 2x[128,512]
    # (2 banks), p_fil = 2x[128,512] (2 banks, QKV/transpose/out-proj).
    p_sc = ctx.enter_context(tc.tile_pool(name="p_sc", bufs=2, space="PSUM"))
    p_pv = ctx.enter_context(tc.tile_pool(name="p_pv", bufs=2, space="PSUM"))
    p_fil = ctx.enter_context(tc.tile_pool(name="p_fil", bufs=2, space="PSUM"))

    # ---------------- constants / weights ----------------
    # DMA order matters for the pipeline head: bias+wq+first hs tile first
    # so the first projection matmuls can fire ASAP; ow (needed only ~100us
    # in, at the first out-projection) goes last.
    bias_sb = consts.tile([128, 3], F32, tag="bias")
    wq_sb = consts.tile([128, 8, 128], F16, tag="wq")
    wk_sb = consts.tile([128, 8, 128], F16, tag="wk")
    wv_sb = consts.tile([128, 8, 128], F16, tag="wv")
    cpack = consts.tile([128, 144], F16, tag="cpack")
    ow_sb = consts.tile([128, 1024], F16, tag="ow")
    ident = cpack[:, 0:128]
    qb_sb, kb_sb, vb_sb = bias_sb[:, 0:1], bias_sb[:, 1:2], bias_sb[:, 2:3]

    nc.sync.dma_start(bias_sb[:], io["bias3"][:])
    nc.sync.dma_start(wq_sb[:], wq_t.rearrange("(t p) m -> p t m", p=128))
    hst0 = hst_pool.tile([128, 8, 512], F16, tag="hst", name="hst0")
    nc.sync.dma_start(
        hst0[:, 0:4, :], hs_t[0:512, 0:512].rearrange("(t p) n -> p t n", p=128)
    )
    nc.sync.dma_start(
        hst0[:, 4:8, :], hs_t[512:1024, 0:512].rearrange("(t p) n -> p t n", p=128)
    )
    nc.sync.dma_start(wk_sb[:], wk_t.rearrange("(t p) m -> p t m", p=128))
    nc.sync.dma_start(wv_sb[:], wv_t.rearrange("(t p) m -> p t m", p=128))
    nc.sync.dma_start(cpack[:], io["cpack"][:])
    nc.sync.dma_start(ow_sb[:], ow_t[:])

    # persistent activations: feature dim (128 = 2 heads x 64) on partitions
    qt = [persist.tile([128, S], F16, tag=f"qt{b}", name=f"qt{b}") for b in range(B)]
    kt = [persist.tile([128, S], F16, tag=f"kt{b}", name=f"kt{b}") for b in range(B)]
    at = [
        [
            persist.tile([128, 1024], F16, tag=f"at{b}{qb}", name=f"at{b}{qb}")
            for qb in range(2)
        ]
        for b in range(B)
    ]
    # Per-head stationary V tiles; tile index = 128-k-token block.
    # v0[b][:, kt, 0:64] = head-0 values, [:, kt, 64] = ones (denom row 64).
    # v1[b][:, kt, 32] = ones (denom row 32 -- engine partition accesses
    # must be 32-aligned), [:, kt, 64:128] = head-1 values (output rows
    # 64:128), other columns zero.
    v0 = [
        persist.tile([128, 16, 65], F16, tag=f"v0{b}", name=f"v0{b}") for b in range(B)
    ]
    v1 = [
        persist.tile([128, 16, 128], F16, tag=f"v1{b}", name=f"v1{b}")
        for b in range(B)
    ]
    ones_col = cpack[:, 128:144].rearrange("p (a o) -> p a o", o=1)
    # [1, 64] f16 all-ones row: stationary for the K=1 broadcast matmuls
    # that replicate the reciprocal row across 64 partitions.
    ones_row = consts.tile([1, 64], F16, tag="ones_row")
    nc.gpsimd.memset(ones_row[:], 1.0)
    for b in range(B):
        nc.gpsimd.memset(v1[b][:, :, 0:64], 0.0)
        nc.gpsimd.tensor_copy(v0[b][:, :, 64:65], ones_col)
        nc.gpsimd.tensor_copy(v1[b][:, :, 32:33], ones_col)

    # ---------------- building blocks ----------------
    def qkv_block(tb, hst=None):
        """Q^T/K^T/V^T projections + V transposes for one 512-token block."""
        b, tl = tb // 4, tb % 4
        if hst is None:
            hst = hst_pool.tile([128, 8, 512], F16, tag="hst")
            nc.sync.dma_start(
                hst[:],
                hs_t[:, tb * 512:(tb + 1) * 512].rearrange("(t p) n -> p t n", p=128),
            )
        c0 = tl * 512
        # K before Q: attention QK consumes kt tiles progressively, so K
        # columns landing first lets the exp stream ramp sooner.
        for w_sb, b_ap, dest in ((wk_sb, kb_sb, kt[b]), (wq_sb, qb_sb, qt[b])):
            ps = p_fil.tile([128, 512], F32, tag="fil", name="ps")
            for et in range(8):
                nc.tensor.matmul(
                    ps[:], w_sb[:, et, :], hst[:, et, :],
                    start=(et == 0), stop=(et == 7),
                )
            nc.vector.tensor_scalar_add(dest[:, c0:c0 + 512], ps[:], b_ap)
        vps = p_fil.tile([128, 512], F32, tag="fil", name="vps")
        for et in range(8):
            nc.tensor.matmul(
                vps[:], wv_sb[:, et, :], hst[:, et, :],
                start=(et == 0), stop=(et == 7),
            )
        vtt = vt_pool.tile([128, 512], F16, tag="vtt")
        nc.vector.tensor_scalar_add(vtt[:], vps[:], vb_sb)
        for j in range(4):
            ktl = tl * 4 + j
            tps = p_fil.tile([128, 128], F16, tag="fil", name="tps")
            nc.tensor.transpose(tps[:], vtt[:, j * 128:(j + 1) * 128], ident[:])
            nc.vector.tensor_copy(v0[b][:, ktl, 0:64], tps[:, 0:64])
            nc.vector.tensor_copy(v1[b][:, ktl, 64:128], tps[:, 64:128])

    def attn_unit(b, qb, h):
        """Scores + exp + PV accumulation for one (batch, q-block, head).

        Returns the two PSUM PV accumulators (qs = 0, 1)."""
        p0 = h * 64
        qoff = qb * 1024
        v_sb = v0[b] if h == 0 else v1[b]
        mrows = 65 if h == 0 else 128

        def emit_qk(k2):
            sc = p_sc.tile([128, 1024], F32, tag="sc", name="sc")
            for qs in range(2):
                nc.tensor.matmul(
                    sc[:, qs * 512:(qs + 1) * 512],
                    kt[b][p0:p0 + 64, k2 * 128:(k2 + 1) * 128],
                    qt[b][p0:p0 + 64, qoff + qs * 512:qoff + (qs + 1) * 512],
                    start=True, stop=True,
                )
            return sc

        pvq = [
            p_pv.tile([mrows, 512], F32, tag="pv", name=f"pv{qs}") for qs in range(2)
        ]
        sc_next = emit_qk(0)
        for k2 in range(16):
            sc = sc_next
            pt = pt_pool.tile([128, 1024], F16, tag="pt")
            nc.scalar.activation(pt[:], sc[:], EXP, scale=SCALE)
            if k2 < 15:
                sc_next = emit_qk(k2 + 1)
            first, last = k2 == 0, k2 == 15
            for qs in range(2):
                nc.tensor.matmul(
                    pvq[qs][:], v_sb[:, k2, :],
                    pt[:, qs * 512:(qs + 1) * 512], start=first, stop=last,
                )
        return pvq

    def attn_norm(b, qb, pvh0, pvh1):
        """Evacuate both heads' PV accumulators, normalize into at[b][qb].

        Runs at high scheduler priority: the evacuation copies release the
        PV PSUM ring that gates the next unit's PV matmuls (and through the
        pt ring, the exp stream), so they must never queue behind lower-
        urgency DVE work like out-projection evacuations."""
        with tc.high_priority():
            _attn_norm(b, qb, pvh0, pvh1)

    def _attn_norm(b, qb, pvh0, pvh1):
        # pvs layout: cols 0:1024 = head 0 (rows 0:64 values, row 64 denom),
        # cols 1024:2048 = head 1 (row 32 denom, rows 64:128 values).
        pvs = rc_pool.tile([128, 2048], F32, tag="pvs", name="pvs")
        for qs in range(2):
            nc.vector.tensor_copy(pvs[0:65, qs * 512:(qs + 1) * 512], pvh0[qs][:])
            nc.vector.tensor_copy(
                pvs[:, 1024 + qs * 512:1024 + (qs + 1) * 512], pvh1[qs][:]
            )
        # Reciprocal of the 2048 denominators (2 heads x 1024 q). DVE
        # reciprocal costs ~6.3 ns per free-dim element regardless of
        # partition count, so pack the two denominator rows across 128
        # partitions with SBUF->SBUF DMAs (issued on the idle HWDGE sync
        # queue -- latency matters, this chain gates the q-block's
        # out-projection), recip in one DVE op (fp16 out: the denominator
        # scale is a per-(head,q) common factor, 5e-4 is plenty), unpack to
        # a [1,2048] row, then replicate across partitions with K=1
        # ones-stationary matmuls into a PSUM tile from the pv ring.
        dpack = rc_pool.tile([128, 16], F32, tag="dpack", name="dpack")
        nc.sync.dma_start(dpack[0:64, :], pvs[64:65, 0:1024])
        nc.sync.dma_start(dpack[64:128, :], pvs[32:33, 1024:2048])
        rpack = rc_pool.tile([128, 16], F16, tag="rpack", name="rpack")
        with nc.allow_low_precision(reason="softmax denom reciprocal"):
            nc.vector.reciprocal(rpack[:], dpack[:])
        rrow = rc_pool.tile([1, 2048], F16, tag="rrow", name="rrow")
        nc.sync.dma_start(rrow[:], rpack[:])
        for qs in range(2):
            c0 = qs * 512
            bc = p_pv.tile([128, 512], F32, tag="pv", name="bc")
            nc.tensor.matmul(
                bc[0:64, :], ones_row[:], rrow[0:1, c0:c0 + 512],
                start=True, stop=True,
            )
            nc.tensor.matmul(
                bc[64:128, :], ones_row[:], rrow[0:1, 1024 + c0:1024 + c0 + 512],
                start=True, stop=True,
            )
            nc.vector.tensor_mul(
                at[b][qb][0:64, c0:c0 + 512], pvs[0:64, c0:c0 + 512], bc[0:64, :]
            )
            nc.vector.tensor_mul(
                at[b][qb][64:128, c0:c0 + 512],
                pvs[64:128, 1024 + c0:1024 + c0 + 512],
                bc[64:128, :],
            )

    def outproj(b, qb, tail=False):
        """Partial out-projection for one q-block's 1024 tokens.

        tail=True (final q-block only, after the last exp): split the PSUM
        evacuations DVE/ScalarE so they drain in parallel -- ScalarE is
        idle once the exp stream ends, and nothing later queues behind it.
        """
        for tb in range(8):
            t0g = b * 2048 + qb * 1024 + tb * 128
            t0l = tb * 128
            ot = out_pool.tile([128, 1024], F16, tag="outs", name="ot")
            for eb in range(2):
                ops = p_fil.tile([128, 512], F32, tag="fil", name="ops")
                nc.tensor.matmul(
                    ops[:], at[b][qb][:, t0l:t0l + 128],
                    ow_sb[:, eb * 512:(eb + 1) * 512],
                    start=True, stop=True,
                )
                if tail and eb == 1:
                    nc.scalar.copy(ot[:, eb * 512:(eb + 1) * 512], ops[:])
                else:
                    nc.vector.tensor_copy(ot[:, eb * 512:(eb + 1) * 512], ops[:])
            nc.gpsimd.dma_start(out_p[t0g:t0g + 128, :], ot[:])

    # ---------------- pipeline ----------------
    # Emission order = scheduler priority. Attention units lead; QKV blocks
    # and out-projections trail the units they should hide under, becoming
    # filler the PE runs whenever the exp-paced attention work isn't ready.
    qkv_block(0, hst0)
    qkv_block(1)

    pv_h = {}
    pv_h[0] = attn_unit(0, 0, 0)
    qkv_block(2)
    pv_h[1] = attn_unit(0, 0, 1)
    qkv_block(3)
    attn_norm(0, 0, pv_h[0], pv_h[1])
    pv_h[0] = attn_unit(0, 1, 0)
    qkv_block(4)
    pv_h[1] = attn_unit(0, 1, 1)
    qkv_block(5)
    attn_norm(0, 1, pv_h[0], pv_h[1])

    pv_h[0] = attn_unit(1, 0, 0)
    qkv_block(6)
    pv_h[1] = attn_unit(1, 0, 1)
    qkv_block(7)
    attn_norm(1, 0, pv_h[0], pv_h[1])
    pv_h[0] = attn_unit(1, 1, 0)
    outproj(0, 0)
    outproj(0, 1)
    pv_h[1] = attn_unit(1, 1, 1)
    outproj(1, 0)
    attn_norm(1, 1, pv_h[0], pv_h[1])
    outproj(1, 1, tail=True)

    if DEBUG:
        for b in range(B):
            nc.sync.dma_start(io["dbg_qt"][:, b * S:(b + 1) * S], qt[b][:])
            nc.sync.dma_start(io["dbg_kt"][:, b * S:(b + 1) * S], kt[b][:])
            for qb in range(2):
                nc.sync.dma_start(
                    io["dbg_at"][:, b * S + qb * 1024:b * S + (qb + 1) * 1024],
                    at[b][qb][:],
                )
        nc.sync.dma_start(
            io["dbg_v00"][:], v0[0][:].rearrange("p a b -> p (a b)")
        )


def _get_program():
    if "nc" in _CACHE:
        return _CACHE["nc"]
    from contextlib import ExitStack

    nc = bacc.Bacc("TRN2", target_bir_lowering=False, debug=False,
                   num_devices=NCORE)
    io = {
        "hs_t": nc.dram_tensor("hs_t", [E, T], F16, kind="ExternalInput").ap(),
        "wq_t": nc.dram_tensor("wq_t", [E, FPC], F16, kind="ExternalInput").ap(),
        "wk_t": nc.dram_tensor("wk_t", [E, FPC], F16, kind="ExternalInput").ap(),
        "wv_t": nc.dram_tensor("wv_t", [E, FPC], F16, kind="ExternalInput").ap(),
        "ow_t": nc.dram_tensor("ow_t", [FPC, E], F16, kind="ExternalInput").ap(),
        "bias3": nc.dram_tensor("bias3", [FPC, 3], F32, kind="ExternalInput").ap(),
        "cpack": nc.dram_tensor("cpack", [128, 144], F16, kind="ExternalInput").ap(),
        "out_p": nc.dram_tensor("out_p", [T, E], F16, kind="ExternalOutput").ap(),
    }
    if DEBUG:
        io["dbg_qt"] = nc.dram_tensor("dbg_qt", [128, T], F16, kind="ExternalOutput").ap()
        io["dbg_kt"] = nc.dram_tensor("dbg_kt", [128, T], F16, kind="ExternalOutput").ap()
        io["dbg_at"] = nc.dram_tensor("dbg_at", [128, T], F16, kind="ExternalOutput").ap()
        io["dbg_v00"] = nc.dram_tensor("dbg_v00", [128, 16 * 65], F16, kind="ExternalOutput").ap()
    with tile.TileContext(nc) as tc:
        with ExitStack() as ctx:
            _build(ctx, tc, io)
    nc.compile()
    _CACHE["nc"] = nc
    return nc


def kernel(hidden_states, q_w, q_b, k_w, k_b, v_w, v_b, o_w, o_b):
    global LAST_RESULT
    nc = _get_program()

    f32c = lambda a: np.ascontiguousarray(a, dtype=np.float32)
    f16c = lambda a: np.ascontiguousarray(a, dtype=np.float16)
    hs_t = f16c(np.asarray(hidden_states, dtype=np.float32).reshape(T, E).T)
    in_maps = []
    for c in range(NCORE):
        sl = slice(c * FPC, (c + 1) * FPC)
        in_maps.append({
            "hs_t": hs_t,
            "wq_t": f16c(np.asarray(q_w)[sl, :].T),
            "wk_t": f16c(np.asarray(k_w)[sl, :].T),
            "wv_t": f16c(np.asarray(v_w)[sl, :].T),
            "ow_t": f16c(np.asarray(o_w)[:, sl].T),
            "bias3": f32c(np.stack([np.asarray(q_b)[sl], np.asarray(k_b)[sl],
                                     np.asarray(v_b)[sl]], axis=1)),
            "cpack": f16c(np.concatenate([np.eye(128, dtype=np.float16),
                                          np.ones((128, 16), np.float16)], axis=1)),
        })

    res = run_bass_kernel_spmd(nc, in_maps, list(range(NCORE)), trace=TRACE)
    LAST_RESULT = res
    out = res.results[0]["out_p"].astype(np.float64)
    for c in range(1, NCORE):
        out += res.results[c]["out_p"]
    out += np.asarray(o_b, dtype=np.float64)
    return out.reshape(B, S, E).astype(np.float32)


# revision 16
# speedup vs baseline: 1.0407x; 1.0222x over previous
"""Trainium2 Bass kernel for a dense multi-head self-attention block.

Computation (matches torch/diffusers Attention with upcast softmax):
    q/k/v = hs @ W.T + b ; per-head scaled QK^T ; softmax ; PV ; out proj.
Shapes: hs [2, 2048, 1024], 16 heads x 64 dim, fp32 in/out.

Sharding: batch*head parallel over 8 cores. Core c owns heads {2c, 2c+1}
(feature slice c*128:(c+1)*128 of E) for both batches. The host
pre-transposes hidden_states to [E, B*S] and pre-slices/transposes the
weights (fp16), so the device never transposes activations. Per core:
  - Q^T/K^T/V^T projections for its 128 features over all 4096 tokens
    (fp16 operands, fp32 PSUM accumulation),
  - V^T is re-tiled to [tokens, features] via PE transposes into per-head
    stationary tiles; an all-ones column rides along so the PV matmul also
    accumulates the softmax denominator,
  - attention in scores^T layout (K @ Q^T), one head at a time per
    (batch, 1024-wide q-block): QK fills a double-buffered [128,1024]
    PSUM score tile per 128-k-token step, exp runs on ScalarE straight
    out of PSUM with the 1/sqrt(d) scale folded in (no max-subtraction:
    scores are O(1) by construction), PV accumulates [*,512] PSUM tiles.
    Head 0's V tile carries values in columns 0:64 + ones in column 64
    (denominator lands on PSUM row 64); head 1's V tile carries ones in
    column 63 + values in columns 64:128 (denominator on row 63, values
    on rows 64:128) so the normalized output lands directly on SBUF
    partitions 64:128 with no partition-shift DMA.
  - softmax normalization: denominators are packed across 128 partitions
    via a DRAM bounce, reciprocal'd in one cheap DVE op, broadcast back
    with DMA broadcast-reads, then fused into the PSUM->SBUF multiply,
  - partial out-projection (contraction over this core's 128 features)
    written as fp16 [4096, 1024]; the host sums the 8 partials + o_b.

Pipelining: the whole kernel is emitted so the Tile dataflow scheduler
overlaps phases. Attention for batch 0 starts as soon as batch 0's QKV
is done; batch 1's QKV projections and all out-projections are emitted
AFTER the attention units they should hide under, so the (in-priority-
order) PE ready-heap treats them as filler for the ~200ns/step the PE
would otherwise idle while ScalarE (the phase drum, ~1.05us per 128k x
1024q exp) catches up. PSUM: score pool 4 banks + PV pool 2 banks +
filler pool (QKV/out-proj/transposes) 2 banks. ScalarE does exp ONLY;
every PSUM evacuation is on DVE; norm-bounce + output DMAs issue from
the otherwise-idle GpSimd queue.
"""

import numpy as np

import concourse.bass as bass
import concourse.mybir as mybir
import concourse.tile as tile
from concourse import bacc
from concourse.bass_utils import run_bass_kernel_spmd

B, S, E = 2, 2048, 1024
H, D = 16, 64
SCALE = D ** -0.5
NCORE = 8
T = B * S              # 4096 tokens
FPC = 128              # features per core (2 heads x 64)
HPC = 2                # heads per core

F32 = mybir.dt.float32
F16 = mybir.dt.float16
EXP = mybir.ActivationFunctionType.Exp

# set by test harness to profile; results stashed in LAST_RESULT
TRACE = False
DEBUG = False
LAST_RESULT = None
_CACHE = {}


def _build(ctx, tc, io):
    nc = tc.nc
    hs_t, wq_t, wk_t, wv_t, ow_t, out_p = (
        io["hs_t"], io["wq_t"], io["wk_t"], io["wv_t"], io["ow_t"], io["out_p"],
    )

    # ---------------- pools ----------------
    consts = ctx.enter_context(tc.tile_pool(name="consts", bufs=1))
    persist = ctx.enter_context(tc.tile_pool(name="persist", bufs=1))
    hst_pool = ctx.enter_context(tc.tile_pool(name="hst", bufs=4))
    vt_pool = ctx.enter_context(tc.tile_pool(name="vt", bufs=3))
    pt_pool = ctx.enter_context(tc.tile_pool(name="pt", bufs=8))
    rc_pool = ctx.enter_context(tc.tile_pool(name="rc", bufs=2))
    out_pool = ctx.enter_context(tc.tile_pool(name="outs", bufs=6))
    # PSUM: 8 banks. p_sc = 2x[128,1024] (4 banks), p_pv = 2x[128,512]
    # (2 banks), p_fil = 2x[128,512] (2 banks, QKV/transpose/out-proj).
    p_sc = ctx.enter_context(tc.tile_pool(name="p_sc", bufs=2, space="PSUM"))
    p_pv = ctx.enter_context(tc.tile_pool(name="p_pv", bufs=2, space="PSUM"))
    p_fil = ctx.enter_context(tc.tile_pool(name="p_fil", bufs=2, space="PSUM"))

    # ---------------- constants / weights ----------------
    # DMA order matters for the pipeline head: bias+wq+first hs tile first
    # so the first projection matmuls can fire ASAP; ow (needed only ~100us
    # in, at the first out-projection) goes last.
    bias_sb = consts.tile([128, 3], F32, tag="bias")
    wq_sb = consts.tile([128, 8, 128], F16, tag="wq")
    wk_sb = consts.tile([128, 8, 128], F16, tag="wk")
    wv_sb = consts.tile([128, 8, 128], F16, tag="wv")
    cpack = consts.tile([128, 144], F16, tag="cpack")
    ow_sb = consts.tile([128, 1024], F16, tag="ow")
    ident = cpack[:, 0:128]
    qb_sb, kb_sb, vb_sb = bias_sb[:, 0:1], bias_sb[:, 1:2], bias_sb[:, 2:3]

    nc.sync.dma_start(bias_sb[:], io["bias3"][:])
    nc.sync.dma_start(wk_sb[:], wk_t.rearrange("(t p) m -> p t m", p=128))
    hst0 = hst_pool.tile([128, 8, 512], F16, tag="hst", name="hst0")
    nc.sync.dma_start(
        hst0[:, 0:4, :], hs_t[0:512, 0:512].rearrange("(t p) n -> p t n", p=128)
    )
    nc.sync.dma_start(wq_sb[:], wq_t.rearrange("(t p) m -> p t m", p=128))
    nc.sync.dma_start(
        hst0[:, 4:8, :], hs_t[512:1024, 0:512].rearrange("(t p) n -> p t n", p=128)
    )
    nc.sync.dma_start(wv_sb[:], wv_t.rearrange("(t p) m -> p t m", p=128))
    nc.sync.dma_start(cpack[:], io["cpack"][:])
    nc.sync.dma_start(ow_sb[:], ow_t[:])

    # persistent activations: feature dim (128 = 2 heads x 64) on partitions
    qt = [persist.tile([128, S], F16, tag=f"qt{b}", name=f"qt{b}") for b in range(B)]
    kt = [persist.tile([128, S], F16, tag=f"kt{b}", name=f"kt{b}") for b in range(B)]
    at = [
        [
            persist.tile([128, 1024], F16, tag=f"at{b}{qb}", name=f"at{b}{qb}")
            for qb in range(2)
        ]
        for b in range(B)
    ]
    # Per-head stationary V tiles; tile index = 128-k-token block.
    # v0[b][:, kt, 0:64] = head-0 values, [:, kt, 64] = ones (denom row 64).
    # v1[b][:, kt, 32] = ones (denom row 32 -- engine partition accesses
    # must be 32-aligned), [:, kt, 64:128] = head-1 values (output rows
    # 64:128), other columns zero.
    v0 = [
        persist.tile([128, 16, 65], F16, tag=f"v0{b}", name=f"v0{b}") for b in range(B)
    ]
    v1 = [
        persist.tile([128, 16, 128], F16, tag=f"v1{b}", name=f"v1{b}")
        for b in range(B)
    ]
    ones_col = cpack[:, 128:144].rearrange("p (a o) -> p a o", o=1)
    # [1, 64] f16 all-ones row: stationary for the K=1 broadcast matmuls
    # that replicate the reciprocal row across 64 partitions.
    ones_row = consts.tile([1, 64], F16, tag="ones_row")
    nc.gpsimd.memset(ones_row[:], 1.0)
    for b in range(B):
        nc.gpsimd.memset(v1[b][:, :, 0:64], 0.0)
        nc.gpsimd.tensor_copy(v0[b][:, :, 64:65], ones_col)
        nc.gpsimd.tensor_copy(v1[b][:, :, 32:33], ones_col)

    # ---------------- building blocks ----------------
    def qkv_block(tb, hst=None):
        """Q^T/K^T/V^T projections + V transposes for one 512-token block."""
        b, tl = tb // 4, tb % 4
        if hst is None:
            hst = hst_pool.tile([128, 8, 512], F16, tag="hst")
            nc.sync.dma_start(
                hst[:],
                hs_t[:, tb * 512:(tb + 1) * 512].rearrange("(t p) n -> p t n", p=128),
            )
        c0 = tl * 512
        # K before Q: attention QK consumes kt tiles progressively, so K
        # columns landing first lets the exp stream ramp sooner.
        for w_sb, b_ap, dest in ((wk_sb, kb_sb, kt[b]), (wq_sb, qb_sb, qt[b])):
            ps = p_fil.tile([128, 512], F32, tag="fil", name="ps")
            for et in range(8):
                nc.tensor.matmul(
                    ps[:], w_sb[:, et, :], hst[:, et, :],
                    start=(et == 0), stop=(et == 7),
                )
            nc.vector.tensor_scalar_add(dest[:, c0:c0 + 512], ps[:], b_ap)
        vps = p_fil.tile([128, 512], F32, tag="fil", name="vps")
        for et in range(8):
            nc.tensor.matmul(
                vps[:], wv_sb[:, et, :], hst[:, et, :],
                start=(et == 0), stop=(et == 7),
            )
        vtt = vt_pool.tile([128, 512], F16, tag="vtt")
        nc.vector.tensor_scalar_add(vtt[:], vps[:], vb_sb)
        for j in range(4):
            ktl = tl * 4 + j
            tps = p_fil.tile([128, 128], F16, tag="fil", name="tps")
            nc.tensor.transpose(tps[:], vtt[:, j * 128:(j + 1) * 128], ident[:])
            nc.vector.tensor_copy(v0[b][:, ktl, 0:64], tps[:, 0:64])
            nc.vector.tensor_copy(v1[b][:, ktl, 64:128], tps[:, 64:128])

    def attn_unit(b, qb, h):
        """Scores + exp + PV accumulation for one (batch, q-block, head).

        Returns the two PSUM PV accumulators (qs = 0, 1)."""
        p0 = h * 64
        qoff = qb * 1024
        v_sb = v0[b] if h == 0 else v1[b]
        mrows = 65 if h == 0 else 128

        def emit_qk(k2):
            sc = p_sc.tile([128, 1024], F32, tag="sc", name="sc")
            for qs in range(2):
                nc.tensor.matmul(
                    sc[:, qs * 512:(qs + 1) * 512],
                    kt[b][p0:p0 + 64, k2 * 128:(k2 + 1) * 128],
                    qt[b][p0:p0 + 64, qoff + qs * 512:qoff + (qs + 1) * 512],
                    start=True, stop=True,
                )
            return sc

        pvq = [
            p_pv.tile([mrows, 512], F32, tag="pv", name=f"pv{qs}") for qs in range(2)
        ]
        sc_next = emit_qk(0)
        for k2 in range(16):
            sc = sc_next
            pt = pt_pool.tile([128, 1024], F16, tag="pt")
            nc.scalar.activation(pt[:], sc[:], EXP, scale=SCALE)
            if k2 < 15:
                sc_next = emit_qk(k2 + 1)
            first, last = k2 == 0, k2 == 15
            for qs in range(2):
                nc.tensor.matmul(
                    pvq[qs][:], v_sb[:, k2, :],
                    pt[:, qs * 512:(qs + 1) * 512], start=first, stop=last,
                )
        return pvq

    def attn_norm(b, qb, pvh0, pvh1):
        """Evacuate both heads' PV accumulators, normalize into at[b][qb].

        Runs at high scheduler priority: the evacuation copies release the
        PV PSUM ring that gates the next unit's PV matmuls (and through the
        pt ring, the exp stream), so they must never queue behind lower-
        urgency DVE work like out-projection evacuations."""
        with tc.high_priority():
            _attn_norm(b, qb, pvh0, pvh1)

    def _attn_norm(b, qb, pvh0, pvh1):
        # pvs layout: cols 0:1024 = head 0 (rows 0:64 values, row 64 denom),
        # cols 1024:2048 = head 1 (row 32 denom, rows 64:128 values).
        pvs = rc_pool.tile([128, 2048], F32, tag="pvs", name="pvs")
        for qs in range(2):
            nc.vector.tensor_copy(pvs[0:65, qs * 512:(qs + 1) * 512], pvh0[qs][:])
            nc.vector.tensor_copy(
                pvs[:, 1024 + qs * 512:1024 + (qs + 1) * 512], pvh1[qs][:]
            )
        # Reciprocal of the 2048 denominators (2 heads x 1024 q). DVE
        # reciprocal costs ~6.3 ns per free-dim element regardless of
        # partition count, so pack the two denominator rows across 128
        # partitions with SBUF->SBUF DMAs (issued on the idle HWDGE sync
        # queue -- latency matters, this chain gates the q-block's
        # out-projection), recip in one DVE op (fp16 out: the denominator
        # scale is a per-(head,q) common factor, 5e-4 is plenty), unpack to
        # a [1,2048] row, then replicate across partitions with K=1
        # ones-stationary matmuls into a PSUM tile from the pv ring.
        dpack = rc_pool.tile([128, 16], F32, tag="dpack", name="dpack")
        nc.sync.dma_start(dpack[0:64, :], pvs[64:65, 0:1024])
        nc.sync.dma_start(dpack[64:128, :], pvs[32:33, 1024:2048])
        rpack = rc_pool.tile([128, 16], F16, tag="rpack", name="rpack")
        with nc.allow_low_precision(reason="softmax denom reciprocal"):
            nc.vector.reciprocal(rpack[:], dpack[:])
        rrow = rc_pool.tile([1, 2048], F16, tag="rrow", name="rrow")
        nc.sync.dma_start(rrow[:], rpack[:])
        for qs in range(2):
            c0 = qs * 512
            # bc lives in the FILLER ring, not the pv ring: its release
            # (after the norm muls) must not gate the next unit's PV
            # matmuls, which would back up the pt ring and stall exp.
            bc = p_fil.tile([128, 512], F32, tag="fil", name="bc")
            nc.tensor.matmul(
                bc[0:64, :], ones_row[:], rrow[0:1, c0:c0 + 512],
                start=True, stop=True,
            )
            nc.tensor.matmul(
                bc[64:128, :], ones_row[:], rrow[0:1, 1024 + c0:1024 + c0 + 512],
                start=True, stop=True,
            )
            nc.vector.tensor_mul(
                at[b][qb][0:64, c0:c0 + 512], pvs[0:64, c0:c0 + 512], bc[0:64, :]
            )
            nc.vector.tensor_mul(
                at[b][qb][64:128, c0:c0 + 512],
                pvs[64:128, 1024 + c0:1024 + c0 + 512],
                bc[64:128, :],
            )

    def outproj(b, qb, tail=False):
        """Partial out-projection for one q-block's 1024 tokens.

        tail=True (final q-block only, after the last exp): split the PSUM
        evacuations DVE/ScalarE so they drain in parallel -- ScalarE is
        idle once the exp stream ends, and nothing later queues behind it.
        """
        for tb in range(8):
            t0g = b * 2048 + qb * 1024 + tb * 128
            t0l = tb * 128
            ot = out_pool.tile([128, 1024], F16, tag="outs", name="ot")
            for eb in range(2):
                ops = p_fil.tile([128, 512], F32, tag="fil", name="ops")
                nc.tensor.matmul(
                    ops[:], at[b][qb][:, t0l:t0l + 128],
                    ow_sb[:, eb * 512:(eb + 1) * 512],
                    start=True, stop=True,
                )
                if tail and eb == 1:
                    nc.scalar.copy(ot[:, eb * 512:(eb + 1) * 512], ops[:])
                else:
                    nc.vector.tensor_copy(ot[:, eb * 512:(eb + 1) * 512], ops[:])
            nc.gpsimd.dma_start(out_p[t0g:t0g + 128, :], ot[:])

    # ---------------- pipeline ----------------
    # Emission order = scheduler priority. Attention units lead; QKV blocks
    # and out-projections trail the units they should hide under, becoming
    # filler the PE runs whenever the exp-paced attention work isn't ready.
    qkv_block(0, hst0)
    qkv_block(1)

    pv_h = {}
    pv_h[0] = attn_unit(0, 0, 0)
    qkv_block(2)
    pv_h[1] = attn_unit(0, 0, 1)
    qkv_block(3)
    attn_norm(0, 0, pv_h[0], pv_h[1])
    pv_h[0] = attn_unit(0, 1, 0)
    qkv_block(4)
    pv_h[1] = attn_unit(0, 1, 1)
    qkv_block(5)
    attn_norm(0, 1, pv_h[0], pv_h[1])

    pv_h[0] = attn_unit(1, 0, 0)
    qkv_block(6)
    pv_h[1] = attn_unit(1, 0, 1)
    qkv_block(7)
    attn_norm(1, 0, pv_h[0], pv_h[1])
    pv_h[0] = attn_unit(1, 1, 0)
    outproj(0, 0)
    outproj(0, 1)
    pv_h[1] = attn_unit(1, 1, 1)
    outproj(1, 0)
    attn_norm(1, 1, pv_h[0], pv_h[1])
    outproj(1, 1, tail=True)

    if DEBUG:
        for b in range(B):
            nc.sync.dma_start(io["dbg_qt"][:, b * S:(b + 1) * S], qt[b][:])
            nc.sync.dma_start(io["dbg_kt"][:, b * S:(b + 1) * S], kt[b][:])
            for qb in range(2):
                nc.sync.dma_start(
                    io["dbg_at"][:, b * S + qb * 1024:b * S + (qb + 1) * 1024],
                    at[b][qb][:],
                )
        nc.sync.dma_start(
            io["dbg_v00"][:], v0[0][:].rearrange("p a b -> p (a b)")
        )


def _get_program():
    if "nc" in _CACHE:
        return _CACHE["nc"]
    from contextlib import ExitStack

    nc = bacc.Bacc("TRN2", target_bir_lowering=False, debug=False,
                   num_devices=NCORE)
    io = {
        "hs_t": nc.dram_tensor("hs_t", [E, T], F16, kind="ExternalInput").ap(),
        "wq_t": nc.dram_tensor("wq_t", [E, FPC], F16, kind="ExternalInput").ap(),
        "wk_t": nc.dram_tensor("wk_t", [E, FPC], F16, kind="ExternalInput").ap(),
        "wv_t": nc.dram_tensor("wv_t", [E, FPC], F16, kind="ExternalInput").ap(),
        "ow_t": nc.dram_tensor("ow_t", [FPC, E], F16, kind="ExternalInput").ap(),
        "bias3": nc.dram_tensor("bias3", [FPC, 3], F32, kind="ExternalInput").ap(),
        "cpack": nc.dram_tensor("cpack", [128, 144], F16, kind="ExternalInput").ap(),
        "out_p": nc.dram_tensor("out_p", [T, E], F16, kind="ExternalOutput").ap(),
    }
    if DEBUG:
        io["dbg_qt"] = nc.dram_tensor("dbg_qt", [128, T], F16, kind="ExternalOutput").ap()
        io["dbg_kt"] = nc.dram_tensor("dbg_kt", [128, T], F16, kind="ExternalOutput").ap()
        io["dbg_at"] = nc.dram_tensor("dbg_at", [128, T], F16, kind="ExternalOutput").ap()
        io["dbg_v00"] = nc.dram_tensor("dbg_v00", [128, 16 * 65], F16, kind="ExternalOutput").ap()
    with tile.TileContext(nc) as tc:
        with ExitStack() as ctx:
            _build(ctx, tc, io)
    nc.compile()
    _CACHE["nc"] = nc
    return nc


def kernel(hidden_states, q_w, q_b, k_w, k_b, v_w, v_b, o_w, o_b):
    global LAST_RESULT
    nc = _get_program()

    f32c = lambda a: np.ascontiguousarray(a, dtype=np.float32)
    f16c = lambda a: np.ascontiguousarray(a, dtype=np.float16)
    hs_t = f16c(np.asarray(hidden_states, dtype=np.float32).reshape(T, E).T)
    in_maps = []
    for c in range(NCORE):
        sl = slice(c * FPC, (c + 1) * FPC)
        in_maps.append({
            "hs_t": hs_t,
            "wq_t": f16c(np.asarray(q_w)[sl, :].T),
            "wk_t": f16c(np.asarray(k_w)[sl, :].T),
            "wv_t": f16c(np.asarray(v_w)[sl, :].T),
            "ow_t": f16c(np.asarray(o_w)[:, sl].T),
            "bias3": f32c(np.stack([np.asarray(q_b)[sl], np.asarray(k_b)[sl],
                                     np.asarray(v_b)[sl]], axis=1)),
            "cpack": f16c(np.concatenate([np.eye(128, dtype=np.float16),
                                          np.ones((128, 16), np.float16)], axis=1)),
        })

    res = run_bass_kernel_spmd(nc, in_maps, list(range(NCORE)), trace=TRACE)
    LAST_RESULT = res
    out = res.results[0]["out_p"].astype(np.float64)
    for c in range(1, NCORE):
        out += res.results[c]["out_p"]
    out += np.asarray(o_b, dtype=np.float64)
    return out.reshape(B, S, E).astype(np.float32)
